# revision 1
# baseline (speedup 1.0000x reference)
"""DGNN (2-hop temporal GNN message passing) Trainium2 kernel.

Strategy (pure data-parallel over events, 8 cores, 512 events/core):

Math: since softmax weights sum to 1 and relu(s*x) = s*relu(x) for s>0,
    x_n_one  = (sum_h s1 * one_hop) @ w2.T + b2
    x_n_two  = (sum_k s2 * two_hop) @ w2.T + b2          (20x less matmul work)
    sum_h s1*relu(pre_h) = sum_h relu(s1*pre_h)           (fold s1 upstream)
so the kernel:
  1. computes s1 = softmax_h(decay1), p2 = s1 * softmax_k(decay2) on-chip,
  2. reduces two_hop [204800,128] over k with PE matmuls against block-diag
     weight tiles (lhsT = data tile [100,128], rhs = [100,5] block-diag of p2)
     giving transposed, s1-scaled agg2T [128feat, cols=(e,h)] directly,
  3. z = relu(w1@oh_sT + w2@agg2T + (b1+b2) x s1row) accumulated in PSUM,
  4. zagg = segmented sum_h z ; agg1 = segmented sum_h oh_sT,
  5. out.T = w3@relu(w1@selfT + w2@agg1 + b12) + w4@zagg + b34.
All feature-major (transposed) layouts so PE contracts the feature dim.
"""
import os, sys
sys.path.insert(0, "/opt/trn_rl_repo")
import numpy as np
import concourse.bass as bass
import concourse.mybir as mybir
import concourse.tile as tile
from concourse import bacc
from concourse.bass_utils import run_bass_kernel_spmd

F32 = mybir.dt.float32
AX = mybir.AxisListType
OP = mybir.AluOpType
ACTF = mybir.ActivationFunctionType

B = 4096
H = 20
F = 128
NCORES = 8
BC = B // NCORES          # events per core = 512
EB = 16                   # events per block
KR = 128                  # two_hop rows per matmul tile (contiguous)
TPB = EB * H * H // KR    # 50 tiles per block
MB = 25                   # bd-build batch (tiles per TT op); phase repeats mod 5


def build(bc=BC, use_f32r=False, use_split=False, repeat=1):
    nblk = bc // EB               # blocks
    et = min(128, bc)             # events per softmax tile
    net = bc // et                # number of softmax tiles
    noh = (bc * H) // 128         # one_hop 128-row tiles
    bcols = EB * H                # 320 columns per block
    assert bc % EB == 0 and (bc % 128 == 0 or bc <= 128) and (bc * H) % 128 == 0

    nc = bacc.Bacc("TRN2", target_bir_lowering=False, debug=False)

    FR = mybir.dt.float32r if use_f32r else F32

    def asf32(ap):
        return ap.bitcast(F32) if use_f32r else ap

    d_self = nc.dram_tensor("self_feat", [bc, F], F32, kind="ExternalInput")
    d_oh = nc.dram_tensor("one_hop", [bc * H, F], F32, kind="ExternalInput")
    d_th = nc.dram_tensor("two_hop", [bc * H * H, F], F32, kind="ExternalInput")
    d_et = nc.dram_tensor("e_time", [bc, 1], F32, kind="ExternalInput")
    d_ht = nc.dram_tensor("his_time", [bc, H], F32, kind="ExternalInput")
    d_hh = nc.dram_tensor("his_his_time", [bc, H * H], F32, kind="ExternalInput")
    d_w = [nc.dram_tensor(f"w{i}", [F, F], F32, kind="ExternalInput") for i in (1, 2, 3, 4)]
    d_b = [nc.dram_tensor(f"b{i}", [1, F], F32, kind="ExternalInput") for i in (1, 2, 3, 4)]
    d_delta = nc.dram_tensor("delta1", [1, 1], F32, kind="ExternalInput")
    d_id = nc.dram_tensor("ident", [F, F], F32, kind="ExternalInput")
    d_mask = nc.dram_tensor("maskblk", [KR, MB * 8], F32, kind="ExternalInput")
    d_out = nc.dram_tensor("out", [bc, F], F32, kind="ExternalOutput")
    d_s1scr = nc.dram_tensor("s1_scratch", [bc, H], FR)
    d_p2scr = nc.dram_tensor("p2_scratch", [bc, H * H], F32)

    with tile.TileContext(nc) as tc:
        with (
            tc.tile_pool(name="const", bufs=1) as cpool,
            tc.tile_pool(name="soft", bufs=2) as soft,
            tc.tile_pool(name="ohin", bufs=4) as ohin,
            tc.tile_pool(name="xin", bufs=2) as xin,
            tc.tile_pool(name="bdp", bufs=2) as bdp,
            tc.tile_pool(name="a2p", bufs=2) as a2p,
            tc.tile_pool(name="zp", bufs=2) as zp,
            tc.tile_pool(name="ps_a", bufs=2, space="PSUM") as ps_a,
            tc.tile_pool(name="ps_z", bufs=2, space="PSUM") as ps_z,
            tc.tile_pool(name="ps_t", bufs=2, space="PSUM") as ps_t,
            tc.tile_pool(name="ps_f", bufs=2, space="PSUM") as ps_f,
        ):
            # ---------------- constants ----------------
            ident = cpool.tile([F, F], F32)
            nc.sync.dma_start(ident[:], d_id[:])
            maskblk = cpool.tile([KR, MB * 8], F32)
            nc.sync.dma_start(maskblk[:], d_mask[:])

            w_t, wT = [], []
            for i in range(4):
                w = cpool.tile([F, F], F32, tag=f"w{i}")
                nc.sync.dma_start(w[:], d_w[i][:])
                w_t.append(w)
                pt = ps_t.tile([F, F], F32, tag="pst")
                nc.tensor.transpose(pt[:], w[:], ident[:])
                wt = cpool.tile([F, F], FR if i < 2 else F32, tag=f"wT{i}")
                nc.scalar.copy(wt[:], pt[:])
                wT.append(wt)
            w1T, w2T, w3T, w4T = wT

            brow = []
            for i in range(4):
                bt = cpool.tile([1, F], F32, tag=f"b{i}")
                nc.sync.dma_start(bt[:], d_b[i][:])
                brow.append(bt)
            b12row = cpool.tile([1, F], FR)
            nc.vector.tensor_add(b12row[:], brow[0][:], brow[1][:])
            b34row = cpool.tile([1, F], F32)
            nc.vector.tensor_add(b34row[:], brow[2][:], brow[3][:])

            d_col = cpool.tile([128, 1], F32)
            nc.sync.dma_start(
                d_col[:],
                bass.AP(tensor=d_delta[:].tensor, offset=d_delta[:].offset,
                        ap=[[0, 128], [1, 1]]))
            ones_row = cpool.tile([1, bc], F32)
            nc.vector.memset(ones_row[:], 1.0)
            zrow = cpool.tile([1, F], mybir.dt.bfloat16)
            nc.vector.memset(zrow[:], 0.0)
            zcols = cpool.tile([1, EB * H], mybir.dt.bfloat16)
            nc.vector.memset(zcols[:], 0.0)

            # ---------------- softmaxes ----------------
            for T in range(net):
                ev = slice(T * et, (T + 1) * et)
                et_t = soft.tile([et, 1], F32, tag="et")
                ht_t = soft.tile([et, H], F32, tag="ht")
                hh_t = soft.tile([et, H * H], F32, tag="hh")
                nc.sync.dma_start(et_t[:], d_et[ev, :])
                nc.sync.dma_start(ht_t[:], d_ht[ev, :])
                nc.sync.dma_start(hh_t[:], d_hh[ev, :])
                dcol = d_col[0:et, :]

                # s1 = softmax_h( delta*(his - e_time) )
                ed = soft.tile([et, 1], F32, tag="ed")
                nc.vector.tensor_scalar_mul(ed[:], et_t[:], dcol)
                u1 = soft.tile([et, H], F32, tag="u1")
                nc.vector.scalar_tensor_tensor(
                    out=u1[:], in0=ht_t[:], scalar=dcol,
                    in1=ed[:].to_broadcast((et, H)),
                    op0=OP.mult, op1=OP.subtract)
                m1 = soft.tile([et, 1], F32, tag="m1")
                nc.vector.tensor_reduce(m1[:], u1[:], axis=AX.X, op=OP.max)
                nm1 = soft.tile([et, 1], F32, tag="nm1")
                nc.vector.tensor_scalar_mul(nm1[:], m1[:], -1.0)
                ex1 = soft.tile([et, H], F32, tag="ex1")
                nc.scalar.activation(ex1[:], u1[:], ACTF.Exp, bias=nm1[:], scale=1.0)
                sm1 = soft.tile([et, 1], F32, tag="sm1")
                nc.vector.tensor_reduce(sm1[:], ex1[:], axis=AX.X, op=OP.add)
                r1 = soft.tile([et, 1], F32, tag="r1")
                nc.vector.reciprocal(r1[:], sm1[:])
                s1_t = soft.tile([et, H], FR, tag="s1t")
                nc.vector.tensor_scalar_mul(s1_t[:], ex1[:], r1[:])
                nc.sync.dma_start(d_s1scr[ev, :], s1_t[:])

                # p2 = s1 * softmax_k( delta*(his_his - his) )
                hd = soft.tile([et, H], F32, tag="hd")
                nc.vector.tensor_scalar_mul(hd[:], ht_t[:], dcol)
                u2 = soft.tile([et, H, H], F32, tag="u2")
                nc.vector.scalar_tensor_tensor(
                    out=u2[:], in0=hh_t[:].rearrange("p (h k) -> p h k", h=H),
                    scalar=dcol, in1=hd[:].to_broadcast((et, H, H)),
                    op0=OP.mult, op1=OP.subtract)
                m2 = soft.tile([et, H], F32, tag="m2")
                nc.vector.tensor_reduce(m2[:], u2[:], axis=AX.X, op=OP.max)
                nc.vector.tensor_sub(u2[:], u2[:], m2[:].to_broadcast((et, H, H)))
                ex2 = u2
                nc.scalar.activation(ex2[:], u2[:], ACTF.Exp)
                sm2 = soft.tile([et, H], F32, tag="sm2")
                nc.vector.tensor_reduce(sm2[:], ex2[:], axis=AX.X, op=OP.add)
                r2 = soft.tile([et, H], F32, tag="r2")
                nc.vector.reciprocal(r2[:], sm2[:])
                r2s = soft.tile([et, H], F32, tag="r2s")
                nc.vector.tensor_mul(r2s[:], r2[:], asf32(s1_t[:]))
                p2 = soft.tile([et, H, H], F32, tag="p2")
                nc.vector.tensor_mul(p2[:], ex2[:], r2s[:].to_broadcast((et, H, H)))

                nc.sync.dma_start(d_p2scr[ev, :], p2[:].rearrange("p a b -> p (a b)"))

            # s1 in flat/col layouts (via DRAM roundtrip)
            s1flat = d_s1scr[:].rearrange("a b -> (a b)")
            s1cols = cpool.tile([128, noh], FR)
            nc.sync.dma_start(s1cols[:], s1flat.rearrange("(u p) -> p u", p=128))
            s1row = cpool.tile([1, bc * H], FR)
            nc.sync.dma_start(s1row[:], s1flat.rearrange("(o f) -> o f", o=1))
            ntile = bc * H * H // KR
            s2p = cpool.tile([KR, ntile], F32)
            p2flat = d_p2scr[:].rearrange("a b -> (a b)")
            nch = max(d for d in (8, 5, 4, 2, 1) if ntile % d == 0)
            gchunk = ntile // nch
            for c in range(nch):
                nc.sync.dma_start(
                    s2p[:, gchunk * c:gchunk * (c + 1)],
                    p2flat[KR * gchunk * c:KR * gchunk * (c + 1)].rearrange(
                        "(t p) -> p t", p=KR))

            # one_hop (scaled by s1 + transposed) is produced inside the
            # block loop, interleaved with the two_hop stream
            ohT = cpool.tile([F, bc * H], FR)

            def emit_oh(u):
                oh_in = ohin.tile([128, F], F32, tag="ohin")
                nc.sync.dma_start(oh_in[:], d_oh[128 * u:128 * (u + 1), :])
                oh_s = ohin.tile([128, F], F32, tag="ohs")
                nc.vector.tensor_scalar_mul(oh_s[:], oh_in[:], asf32(s1cols[:, u:u + 1]))
                pt = ps_t.tile([128, 128], F32, tag="pst")
                nc.tensor.transpose(pt[:], oh_s[:], ident[:])
                nc.scalar.copy(ohT[:, 128 * u:128 * (u + 1)], pt[:])

            # ---------------- main loop over event blocks ----------------
            cw = min(128, bc)
            selfT = cpool.tile([F, bc], F32)
            for c in range(bc // cw):
                sf = ohin.tile([cw, F], F32, tag="sf")
                nc.sync.dma_start(sf[:], d_self[cw * c:cw * (c + 1), :])
                pt = ps_t.tile([F, cw], F32, tag="pst")
                nc.tensor.transpose(pt[:], sf[:], ident[0:cw, 0:cw])
                nc.scalar.copy(selfT[:, cw * c:cw * (c + 1)], pt[:])

            zagg = cpool.tile([F, bc], F32)
            agg1 = cpool.tile([F, bc], F32)
            rep_ctx = tc.For_i(0, repeat, 1) if repeat > 1 else None
            if rep_ctx is not None:
                rep_ctx.__enter__()
            for b in range(nblk):
                # two_hop block: 6400 rows as 50 [128,128] lhsT tiles, contiguous
                xb = xin.tile([KR, TPB * F], F32, tag="xb")
                rb = b * EB * H * H
                for i in range(10):
                    nc.sync.dma_start(
                        xb[:, 640 * i:640 * (i + 1)].rearrange(
                            "p (r f) -> p r f", f=F),
                        d_th[rb + 640 * i: rb + 640 * (i + 1), :].rearrange(
                            "(r p) f -> p r f", p=KR))

                for u in range(-(-(noh * b) // nblk), -(-(noh * (b + 1)) // nblk)):
                    emit_oh(u)

                # block-diag weights for the k-reduction (8 cols per tile)
                bd = bdp.tile([KR, TPB * 8], F32, tag="bd")
                for h in range(2):
                    nc.vector.tensor_tensor(
                        out=bd[:, MB * 8 * h:MB * 8 * (h + 1)].rearrange(
                            "p (t j) -> p t j", j=8),
                        in0=maskblk[:].rearrange("p (t j) -> p t j", j=8),
                        in1=s2p[:, TPB * b + MB * h:TPB * b + MB * (h + 1)
                                ].to_broadcast((KR, MB, 8)),
                        op=OP.mult)

                # k-reduction matmuls: psum cols = s1*agg2 (feature-major)
                pa = ps_a.tile([F, bcols], F32, tag="pa")
                nc.tensor.matmul(pa[:], zrow[:], zcols[:], start=True, stop=False)
                for i in range(TPB):
                    g0 = (KR * i) // H
                    J = min(8, bcols - g0)
                    base = i * F
                    last = i == TPB - 1
                    if use_split:
                        nc.tensor.matmul(
                            pa[0:64, g0:g0 + J], xb[:, base:base + 64],
                            bd[:, 8 * i:8 * i + J], start=False, stop=False,
                            tile_position=(0, 0), skip_group_check=True)
                        nc.tensor.matmul(
                            pa[64:128, g0:g0 + J], xb[:, base + 64:base + F],
                            bd[:, 8 * i:8 * i + J], start=False, stop=False,
                            tile_position=(0, 64), skip_group_check=True)
                    else:
                        nc.tensor.matmul(
                            pa[:, g0:g0 + J], xb[:, base:base + F],
                            bd[:, 8 * i:8 * i + J], start=False, stop=last)
                if use_split:
                    nc.tensor.matmul(pa[:], zrow[:], zcols[:], start=False, stop=True)
                a2 = a2p.tile([F, bcols], FR, tag="a2")
                nc.scalar.copy(a2[:], pa[:])

                # z = relu(w1 @ ohT + w2 @ agg2T + b12 x s1row)
                pz = ps_z.tile([F, bcols], F32, tag="pz")
                cs = slice(bcols * b, bcols * (b + 1))
                nc.tensor.matmul(pz[:], w1T[:], ohT[:, cs],
                                 start=True, stop=False)
                nc.tensor.matmul(pz[:], w2T[:], a2[:],
                                 start=False, stop=False)
                nc.tensor.matmul(pz[:], b12row[:], s1row[:, cs],
                                 start=False, stop=True)
                zs = zp.tile([F, bcols], F32, tag="zs")
                nc.scalar.activation(zs[:], pz[:], ACTF.Relu)

                # segmented sums over h
                nc.vector.tensor_reduce(
                    zagg[:, EB * b:EB * (b + 1)],
                    zs[:].rearrange("p (e h) -> p e h", h=H),
                    axis=AX.X, op=OP.add)
                nc.vector.tensor_reduce(
                    agg1[:, EB * b:EB * (b + 1)],
                    asf32(ohT[:, cs]).rearrange("p (e h) -> p e h", h=H),
                    axis=AX.X, op=OP.add)

            if rep_ctx is not None:
                rep_ctx.__exit__(None, None, None)

            # ---------------- self path + final ----------------
            pxs = ps_f.tile([F, bc], F32, tag="pf")
            nc.tensor.matmul(pxs[:], asf32(w1T[:]), selfT[:], start=True, stop=False)
            nc.tensor.matmul(pxs[:], asf32(w2T[:]), agg1[:], start=False, stop=False)
            nc.tensor.matmul(pxs[:], asf32(b12row[:]), ones_row[:], start=False, stop=True)
            xs = cpool.tile([F, bc], F32)
            nc.scalar.activation(xs[:], pxs[:], ACTF.Relu)

            po = ps_f.tile([F, bc], F32, tag="pf")
            nc.tensor.matmul(po[:], w3T[:], xs[:], start=True, stop=False)
            nc.tensor.matmul(po[:], w4T[:], zagg[:], start=False, stop=False)
            nc.tensor.matmul(po[:], b34row[:], ones_row[:], start=False, stop=True)
            outT = cpool.tile([F, bc], F32)
            nc.vector.tensor_copy(outT[:], po[:])

            for c in range(bc // cw):
                pt = ps_t.tile([cw, F], F32, tag="pst")
                nc.tensor.transpose(pt[:], outT[:, cw * c:cw * (c + 1)], ident[:])
                ob = ohin.tile([cw, F], F32, tag="ob")
                nc.vector.tensor_copy(ob[:], pt[:])
                nc.sync.dma_start(d_out[cw * c:cw * (c + 1), :], ob[:])

    nc.compile()
    return nc


def make_const_inputs():
    ident = np.eye(F, dtype=np.float32)
    # maskblk[p, 8*i + j] = 1 iff row 128*i + p belongs to group g0(i) + j
    maskblk = np.zeros((KR, MB * 8), dtype=np.float32)
    for i in range(MB):
        g0 = (KR * i) // H
        for p in range(KR):
            j = (KR * i + p) // H - g0
            if j < 8:
                maskblk[p, 8 * i + j] = 1.0
    return ident, maskblk


_NC_CACHE = {}


USE_F32R = os.environ.get("DGNN_F32R", "1") == "1"
USE_SPLIT = os.environ.get("DGNN_SPLIT", "1") == "1"


def _get_nc(bc=BC, use_f32r=None, use_split=None):
    if use_f32r is None:
        use_f32r = USE_F32R
    if use_split is None:
        use_split = USE_SPLIT
    key = (bc, use_f32r, use_split)
    if key not in _NC_CACHE:
        _NC_CACHE[key] = build(bc, use_f32r, use_split)
    return _NC_CACHE[key]


def kernel(self_feat, one_hop_feat, two_hop_feat, e_time, his_time,
           his_his_time, w1, b1, w2, b2, w3, b3, w4, b4, delta1):
    self_feat = np.ascontiguousarray(np.asarray(self_feat, dtype=np.float32))
    one_hop_feat = np.ascontiguousarray(np.asarray(one_hop_feat, dtype=np.float32))
    two_hop_feat = np.ascontiguousarray(np.asarray(two_hop_feat, dtype=np.float32))
    e_time = np.asarray(e_time, dtype=np.float32).reshape(B, 1)
    his_time = np.ascontiguousarray(np.asarray(his_time, dtype=np.float32))
    his_his_time = np.asarray(his_his_time, dtype=np.float32).reshape(B, H * H)
    ident, maskblk = make_const_inputs()
    shared = {
        "w1": np.asarray(w1, np.float32), "w2": np.asarray(w2, np.float32),
        "w3": np.asarray(w3, np.float32), "w4": np.asarray(w4, np.float32),
        "b1": np.asarray(b1, np.float32).reshape(1, F),
        "b2": np.asarray(b2, np.float32).reshape(1, F),
        "b3": np.asarray(b3, np.float32).reshape(1, F),
        "b4": np.asarray(b4, np.float32).reshape(1, F),
        "delta1": np.asarray(delta1, np.float32).reshape(1, 1),
        "ident": ident, "maskblk": maskblk,
    }
    in_maps = []
    for i in range(NCORES):
        ev = slice(i * BC, (i + 1) * BC)
        r1 = slice(i * BC * H, (i + 1) * BC * H)
        r2 = slice(i * BC * H * H, (i + 1) * BC * H * H)
        in_maps.append(dict(
            self_feat=self_feat[ev], one_hop=one_hop_feat[r1],
            two_hop=two_hop_feat[r2], e_time=e_time[ev],
            his_time=his_time[ev], his_his_time=his_his_time[ev], **shared))
    nc = _get_nc()
    res = run_bass_kernel_spmd(nc, in_maps, core_ids=list(range(NCORES)))
    return np.concatenate([res.results[i]["out"] for i in range(NCORES)], axis=0)



# revision 15
# speedup vs baseline: 1.3981x; 1.3981x over previous
"""DGNN (2-hop temporal GNN message passing) Trainium2 kernel.

Strategy (pure data-parallel over events, 8 cores, 512 events/core):

Math: since softmax weights sum to 1 and relu(s*x) = s*relu(x) for s>0,
    x_n_one  = (sum_h s1 * one_hop) @ w2.T + b2
    x_n_two  = (sum_k s2 * two_hop) @ w2.T + b2          (20x less matmul work)
    sum_h s1*relu(pre_h) = sum_h relu(s1*pre_h)           (fold s1 upstream)
so the kernel:
  1. computes s1 = softmax_h(decay1), p2 = s1 * softmax_k(decay2) on-chip,
  2. reduces two_hop [204800,128] over k with PE matmuls against block-diag
     weight tiles (lhsT = data tile [100,128], rhs = [100,5] block-diag of p2)
     giving transposed, s1-scaled agg2T [128feat, cols=(e,h)] directly,
  3. z = relu(w1@oh_sT + w2@agg2T + (b1+b2) x s1row) accumulated in PSUM,
  4. zagg = segmented sum_h z ; agg1 = segmented sum_h oh_sT,
  5. out.T = w3@relu(w1@selfT + w2@agg1 + b12) + w4@zagg + b34.
All feature-major (transposed) layouts so PE contracts the feature dim.
"""
import os, sys
sys.path.insert(0, "/opt/trn_rl_repo")
import numpy as np
import concourse.bass as bass
import concourse.mybir as mybir
import concourse.tile as tile
from concourse import bacc
from concourse.bass_utils import run_bass_kernel_spmd

F32 = mybir.dt.float32
AX = mybir.AxisListType
OP = mybir.AluOpType
ACTF = mybir.ActivationFunctionType

B = 4096
H = 20
F = 128
NCORES = 8
BC = B // NCORES          # events per core = 512
EB = 16                   # events per block
KR = 128                  # two_hop rows per matmul tile (contiguous)
TPB = EB * H * H // KR    # 50 tiles per block
MB = 25                   # bd-build batch (tiles per TT op); phase repeats mod 5


def build(bc=BC, use_f32r=False, use_split=False, repeat=1, use_eng2=False):
    nblk = bc // EB               # blocks
    et = min(128, bc)             # events per softmax tile
    net = bc // et                # number of softmax tiles
    noh = (bc * H) // 128         # one_hop 128-row tiles
    bcols = EB * H                # 320 columns per block
    assert bc % EB == 0 and (bc % 128 == 0 or bc <= 128) and (bc * H) % 128 == 0

    nc = bacc.Bacc("TRN2", target_bir_lowering=False, debug=False)

    FR = mybir.dt.float32r if use_f32r else F32

    def asf32(ap):
        return ap.bitcast(F32) if use_f32r else ap

    d_self = nc.dram_tensor("self_feat", [bc, F], F32, kind="ExternalInput")
    d_oh = nc.dram_tensor("one_hop", [bc * H, F], F32, kind="ExternalInput")
    d_th = nc.dram_tensor("two_hop", [bc * H * H, F], F32, kind="ExternalInput")
    d_et = nc.dram_tensor("e_time", [bc, 1], F32, kind="ExternalInput")
    d_ht = nc.dram_tensor("his_time", [bc, H], F32, kind="ExternalInput")
    d_hh = nc.dram_tensor("his_his_time", [bc, H * H], F32, kind="ExternalInput")
    d_w = [nc.dram_tensor(f"w{i}", [F, F], F32, kind="ExternalInput") for i in (1, 2, 3, 4)]
    d_b = [nc.dram_tensor(f"b{i}", [1, F], F32, kind="ExternalInput") for i in (1, 2, 3, 4)]
    d_delta = nc.dram_tensor("delta1", [1, 1], F32, kind="ExternalInput")
    d_id = nc.dram_tensor("ident", [F, F], F32, kind="ExternalInput")
    d_mask = nc.dram_tensor("maskblk", [KR, MB * 8], F32, kind="ExternalInput")
    d_out = nc.dram_tensor("out", [bc, F], F32, kind="ExternalOutput")
    d_s1scr = nc.dram_tensor("s1_scratch", [bc, H], FR)
    d_p2scr = nc.dram_tensor("p2_scratch", [bc, H * H], F32)

    with tile.TileContext(nc) as tc:
        with (
            tc.tile_pool(name="const", bufs=1) as cpool,
            tc.tile_pool(name="soft", bufs=2) as soft,
            tc.tile_pool(name="ohin", bufs=4) as ohin,
            tc.tile_pool(name="xin", bufs=2) as xin,
            tc.tile_pool(name="bdp", bufs=2) as bdp,
            tc.tile_pool(name="a2p", bufs=2) as a2p,
            tc.tile_pool(name="zp", bufs=2) as zp,
            tc.tile_pool(name="ps_a", bufs=2, space="PSUM") as ps_a,
            tc.tile_pool(name="ps_z", bufs=2, space="PSUM") as ps_z,
            tc.tile_pool(name="ps_t", bufs=2, space="PSUM") as ps_t,
            tc.tile_pool(name="ps_f", bufs=2, space="PSUM") as ps_f,
        ):
            # ---------------- constants ----------------
            ident = cpool.tile([F, F], F32)
            nc.sync.dma_start(ident[:], d_id[:])
            maskblk = cpool.tile([KR, MB * 8], F32)
            nc.sync.dma_start(maskblk[:], d_mask[:])

            w_t, wT = [], []
            for i in range(4):
                w = cpool.tile([F, F], F32, tag=f"w{i}")
                nc.sync.dma_start(w[:], d_w[i][:])
                w_t.append(w)
                pt = ps_t.tile([F, F], F32, tag="pst")
                nc.tensor.transpose(pt[:], w[:], ident[:])
                wt = cpool.tile([F, F], FR if i < 2 else F32, tag=f"wT{i}")
                nc.scalar.copy(wt[:], pt[:])
                wT.append(wt)
            w1T, w2T, w3T, w4T = wT

            brow = []
            for i in range(4):
                bt = cpool.tile([1, F], F32, tag=f"b{i}")
                nc.sync.dma_start(bt[:], d_b[i][:])
                brow.append(bt)
            b12row = cpool.tile([1, F], FR)
            nc.vector.tensor_add(b12row[:], brow[0][:], brow[1][:])
            b34row = cpool.tile([1, F], F32)
            nc.vector.tensor_add(b34row[:], brow[2][:], brow[3][:])

            d_col = cpool.tile([128, 1], F32)
            nc.sync.dma_start(
                d_col[:],
                bass.AP(tensor=d_delta[:].tensor, offset=d_delta[:].offset,
                        ap=[[0, 128], [1, 1]]))
            ones_row = cpool.tile([1, bc], F32)
            nc.vector.memset(ones_row[:], 1.0)
            zrow = cpool.tile([1, F], mybir.dt.bfloat16)
            nc.vector.memset(zrow[:], 0.0)
            zcols = cpool.tile([1, EB * H], mybir.dt.bfloat16)
            nc.vector.memset(zcols[:], 0.0)

            # ---------------- softmaxes ----------------
            # under use_eng2 the softmax-phase DMAs ride the Act queue so the
            # SP queue can start streaming two_hop immediately
            seng = nc.scalar if use_eng2 else nc.sync
            for T in range(net):
                ev = slice(T * et, (T + 1) * et)
                et_t = soft.tile([et, 1], F32, tag="et")
                ht_t = soft.tile([et, H], F32, tag="ht")
                hh_t = soft.tile([et, H * H], F32, tag="hh")
                seng.dma_start(et_t[:], d_et[ev, :])
                seng.dma_start(ht_t[:], d_ht[ev, :])
                seng.dma_start(hh_t[:], d_hh[ev, :])
                dcol = d_col[0:et, :]

                # s1 = softmax_h( delta*(his - e_time) )
                ed = soft.tile([et, 1], F32, tag="ed")
                nc.vector.tensor_scalar_mul(ed[:], et_t[:], dcol)
                u1 = soft.tile([et, H], F32, tag="u1")
                nc.vector.scalar_tensor_tensor(
                    out=u1[:], in0=ht_t[:], scalar=dcol,
                    in1=ed[:].to_broadcast((et, H)),
                    op0=OP.mult, op1=OP.subtract)
                m1 = soft.tile([et, 1], F32, tag="m1")
                nc.vector.tensor_reduce(m1[:], u1[:], axis=AX.X, op=OP.max)
                nm1 = soft.tile([et, 1], F32, tag="nm1")
                nc.vector.tensor_scalar_mul(nm1[:], m1[:], -1.0)
                ex1 = soft.tile([et, H], F32, tag="ex1")
                nc.scalar.activation(ex1[:], u1[:], ACTF.Exp, bias=nm1[:], scale=1.0)
                sm1 = soft.tile([et, 1], F32, tag="sm1")
                nc.vector.tensor_reduce(sm1[:], ex1[:], axis=AX.X, op=OP.add)
                r1 = soft.tile([et, 1], F32, tag="r1")
                nc.vector.reciprocal(r1[:], sm1[:])
                s1_t = soft.tile([et, H], FR, tag="s1t")
                nc.vector.tensor_scalar_mul(s1_t[:], ex1[:], r1[:])
                seng.dma_start(d_s1scr[ev, :], s1_t[:])

                # p2 = s1 * softmax_k( delta*(his_his - his) )
                hd = soft.tile([et, H], F32, tag="hd")
                nc.vector.tensor_scalar_mul(hd[:], ht_t[:], dcol)
                u2 = soft.tile([et, H, H], F32, tag="u2")
                nc.vector.scalar_tensor_tensor(
                    out=u2[:], in0=hh_t[:].rearrange("p (h k) -> p h k", h=H),
                    scalar=dcol, in1=hd[:].to_broadcast((et, H, H)),
                    op0=OP.mult, op1=OP.subtract)
                m2 = soft.tile([et, H], F32, tag="m2")
                nc.vector.tensor_reduce(m2[:], u2[:], axis=AX.X, op=OP.max)
                nc.vector.tensor_sub(u2[:], u2[:], m2[:].to_broadcast((et, H, H)))
                ex2 = u2
                nc.scalar.activation(ex2[:], u2[:], ACTF.Exp)
                sm2 = soft.tile([et, H], F32, tag="sm2")
                nc.vector.tensor_reduce(sm2[:], ex2[:], axis=AX.X, op=OP.add)
                r2 = soft.tile([et, H], F32, tag="r2")
                nc.vector.reciprocal(r2[:], sm2[:])
                r2s = soft.tile([et, H], F32, tag="r2s")
                nc.vector.tensor_mul(r2s[:], r2[:], asf32(s1_t[:]))
                p2 = soft.tile([et, H, H], F32, tag="p2")
                nc.vector.tensor_mul(p2[:], ex2[:], r2s[:].to_broadcast((et, H, H)))

                seng.dma_start(d_p2scr[ev, :], p2[:].rearrange("p a b -> p (a b)"))

            # s1 in flat/col layouts (via DRAM roundtrip)
            s1flat = d_s1scr[:].rearrange("a b -> (a b)")
            s1cols = cpool.tile([128, noh], FR)
            seng.dma_start(s1cols[:], s1flat.rearrange("(u p) -> p u", p=128))
            s1row = cpool.tile([1, bc * H], FR)
            seng.dma_start(s1row[:], s1flat.rearrange("(o f) -> o f", o=1))
            ntile = bc * H * H // KR
            s2p = cpool.tile([KR, ntile], F32)
            p2flat = d_p2scr[:].rearrange("a b -> (a b)")
            nch = max(d for d in (8, 5, 4, 2, 1) if ntile % d == 0)
            gchunk = ntile // nch
            for c in range(nch):
                seng.dma_start(
                    s2p[:, gchunk * c:gchunk * (c + 1)],
                    p2flat[KR * gchunk * c:KR * gchunk * (c + 1)].rearrange(
                        "(t p) -> p t", p=KR))

            # one_hop (scaled by s1 + transposed) is produced inside the
            # block loop, interleaved with the two_hop stream
            ohT = cpool.tile([F, bc * H], FR)

            def emit_oh(u):
                oh_in = ohin.tile([128, F], F32, tag="ohin")
                nc.sync.dma_start(oh_in[:], d_oh[128 * u:128 * (u + 1), :])
                oh_s = ohin.tile([128, F], F32, tag="ohs")
                nc.vector.tensor_scalar_mul(oh_s[:], oh_in[:], asf32(s1cols[:, u:u + 1]))
                pt = ps_t.tile([128, 128], F32, tag="pst")
                nc.tensor.transpose(pt[:], oh_s[:], ident[:])
                nc.scalar.copy(ohT[:, 128 * u:128 * (u + 1)], pt[:])

            def emit_oh4(q):
                # 4 one_hop tiles per DMA instruction on the Act queue
                nu = min(4, noh - 4 * q)
                oh_in = ohin.tile([128, nu * F], F32, tag="ohin")
                nc.scalar.dma_start(
                    oh_in[:].rearrange("p (r f) -> p r f", f=F),
                    d_oh[512 * q:512 * q + nu * 128, :].rearrange(
                        "(r p) f -> p r f", p=128))
                oh_s = ohin.tile([128, nu * F], F32, tag="ohs")
                nc.vector.tensor_tensor(
                    out=oh_s[:].rearrange("p (r f) -> p r f", f=F),
                    in0=oh_in[:].rearrange("p (r f) -> p r f", f=F),
                    in1=asf32(s1cols[:, 4 * q:4 * q + nu]).to_broadcast(
                        (128, nu, F)),
                    op=OP.mult)
                for j in range(nu):
                    u = 4 * q + j
                    pt = ps_t.tile([128, 128], F32, tag="pst")
                    nc.tensor.transpose(pt[:], oh_s[:, F * j:F * (j + 1)], ident[:])
                    nc.scalar.copy(ohT[:, 128 * u:128 * (u + 1)], pt[:])

            # ---------------- main loop over event blocks ----------------
            cw = min(128, bc)
            selfT = cpool.tile([F, bc], F32)
            for c in range(bc // cw):
                sf = ohin.tile([cw, F], F32, tag="sf")
                seng.dma_start(sf[:], d_self[cw * c:cw * (c + 1), :])
                pt = ps_t.tile([F, cw], F32, tag="pst")
                nc.tensor.transpose(pt[:], sf[:], ident[0:cw, 0:cw])
                nc.scalar.copy(selfT[:, cw * c:cw * (c + 1)], pt[:])

            zagg = cpool.tile([F, bc], F32)
            agg1 = cpool.tile([F, bc], F32)
            rep_ctx = tc.For_i(0, repeat, 1) if repeat > 1 else None
            if rep_ctx is not None:
                rep_ctx.__enter__()
            for b in range(nblk):
                # two_hop block: 6400 rows as 50 [128,128] lhsT tiles, contiguous
                xb = xin.tile([KR, TPB * F], F32, tag="xb")
                rb = b * EB * H * H
                if use_eng2:
                    # two HWDGE queues stream the block concurrently; SP takes
                    # 26/50 tiles, Act 24/50 (Act also carries the oh stream)
                    for eng, r0, r1 in ((nc.sync, 0, 3328), (nc.scalar, 3328, 6400)):
                        eng.dma_start(
                            xb[:, r0:r1].rearrange("p (r f) -> p r f", f=F),
                            d_th[rb + r0: rb + r1, :].rearrange(
                                "(r p) f -> p r f", p=KR))
                else:
                    for i in range(10):
                        nc.sync.dma_start(
                            xb[:, 640 * i:640 * (i + 1)].rearrange(
                                "p (r f) -> p r f", f=F),
                            d_th[rb + 640 * i: rb + 640 * (i + 1), :].rearrange(
                                "(r p) f -> p r f", p=KR))

                if use_eng2:
                    ngrp = -(-noh // 4)
                    for q in range(-(-(ngrp * b) // nblk),
                                   -(-(ngrp * (b + 1)) // nblk)):
                        emit_oh4(q)
                else:
                    for u in range(-(-(noh * b) // nblk),
                                   -(-(noh * (b + 1)) // nblk)):
                        emit_oh(u)

                # block-diag weights for the k-reduction (8 cols per tile)
                bd = bdp.tile([KR, TPB * 8], F32, tag="bd")
                for h in range(2):
                    nc.vector.tensor_tensor(
                        out=bd[:, MB * 8 * h:MB * 8 * (h + 1)].rearrange(
                            "p (t j) -> p t j", j=8),
                        in0=maskblk[:].rearrange("p (t j) -> p t j", j=8),
                        in1=s2p[:, TPB * b + MB * h:TPB * b + MB * (h + 1)
                                ].to_broadcast((KR, MB, 8)),
                        op=OP.mult)

                # k-reduction matmuls: psum cols = s1*agg2 (feature-major)
                pa = ps_a.tile([F, bcols], F32, tag="pa")
                nc.tensor.matmul(pa[:], zrow[:], zcols[:], start=True, stop=False)
                for i in range(TPB):
                    g0 = (KR * i) // H
                    J = min(8, bcols - g0)
                    base = i * F
                    last = i == TPB - 1
                    if use_split:
                        nc.tensor.matmul(
                            pa[0:64, g0:g0 + J], xb[:, base:base + 64],
                            bd[:, 8 * i:8 * i + J], start=False, stop=False,
                            tile_position=(0, 0), skip_group_check=True)
                        nc.tensor.matmul(
                            pa[64:128, g0:g0 + J], xb[:, base + 64:base + F],
                            bd[:, 8 * i:8 * i + J], start=False, stop=False,
                            tile_position=(0, 64), skip_group_check=True)
                    else:
                        nc.tensor.matmul(
                            pa[:, g0:g0 + J], xb[:, base:base + F],
                            bd[:, 8 * i:8 * i + J], start=False, stop=last)
                if use_split:
                    nc.tensor.matmul(pa[:], zrow[:], zcols[:], start=False, stop=True)
                a2 = a2p.tile([F, bcols], FR, tag="a2")
                nc.scalar.copy(a2[:], pa[:])

                # z = relu(w1 @ ohT + w2 @ agg2T + b12 x s1row)
                pz = ps_z.tile([F, bcols], F32, tag="pz")
                cs = slice(bcols * b, bcols * (b + 1))
                nc.tensor.matmul(pz[:], w1T[:], ohT[:, cs],
                                 start=True, stop=False)
                nc.tensor.matmul(pz[:], w2T[:], a2[:],
                                 start=False, stop=False)
                nc.tensor.matmul(pz[:], b12row[:], s1row[:, cs],
                                 start=False, stop=True)
                zs = zp.tile([F, bcols], F32, tag="zs")
                nc.scalar.activation(zs[:], pz[:], ACTF.Relu)

                # segmented sums over h
                nc.vector.tensor_reduce(
                    zagg[:, EB * b:EB * (b + 1)],
                    zs[:].rearrange("p (e h) -> p e h", h=H),
                    axis=AX.X, op=OP.add)
                nc.vector.tensor_reduce(
                    agg1[:, EB * b:EB * (b + 1)],
                    asf32(ohT[:, cs]).rearrange("p (e h) -> p e h", h=H),
                    axis=AX.X, op=OP.add)

            if rep_ctx is not None:
                rep_ctx.__exit__(None, None, None)

            # ---------------- self path + final ----------------
            pxs = ps_f.tile([F, bc], F32, tag="pf")
            nc.tensor.matmul(pxs[:], asf32(w1T[:]), selfT[:], start=True, stop=False)
            nc.tensor.matmul(pxs[:], asf32(w2T[:]), agg1[:], start=False, stop=False)
            nc.tensor.matmul(pxs[:], asf32(b12row[:]), ones_row[:], start=False, stop=True)
            xs = cpool.tile([F, bc], F32)
            nc.scalar.activation(xs[:], pxs[:], ACTF.Relu)

            po = ps_f.tile([F, bc], F32, tag="pf")
            nc.tensor.matmul(po[:], w3T[:], xs[:], start=True, stop=False)
            nc.tensor.matmul(po[:], w4T[:], zagg[:], start=False, stop=False)
            nc.tensor.matmul(po[:], b34row[:], ones_row[:], start=False, stop=True)
            outT = cpool.tile([F, bc], F32)
            nc.vector.tensor_copy(outT[:], po[:])

            for c in range(bc // cw):
                pt = ps_t.tile([cw, F], F32, tag="pst")
                nc.tensor.transpose(pt[:], outT[:, cw * c:cw * (c + 1)], ident[:])
                ob = ohin.tile([cw, F], F32, tag="ob")
                nc.vector.tensor_copy(ob[:], pt[:])
                nc.sync.dma_start(d_out[cw * c:cw * (c + 1), :], ob[:])

    nc.compile()
    return nc


def make_const_inputs():
    ident = np.eye(F, dtype=np.float32)
    # maskblk[p, 8*i + j] = 1 iff row 128*i + p belongs to group g0(i) + j
    maskblk = np.zeros((KR, MB * 8), dtype=np.float32)
    for i in range(MB):
        g0 = (KR * i) // H
        for p in range(KR):
            j = (KR * i + p) // H - g0
            if j < 8:
                maskblk[p, 8 * i + j] = 1.0
    return ident, maskblk


_NC_CACHE = {}


USE_F32R = os.environ.get("DGNN_F32R", "1") == "1"
USE_SPLIT = os.environ.get("DGNN_SPLIT", "1") == "1"
USE_ENG2 = os.environ.get("DGNN_ENG2", "1") == "1"


def _get_nc(bc=BC, use_f32r=None, use_split=None, use_eng2=None):
    if use_f32r is None:
        use_f32r = USE_F32R
    if use_split is None:
        use_split = USE_SPLIT
    if use_eng2 is None:
        use_eng2 = USE_ENG2
    key = (bc, use_f32r, use_split, use_eng2)
    if key not in _NC_CACHE:
        _NC_CACHE[key] = build(bc, use_f32r, use_split, use_eng2=use_eng2)
    return _NC_CACHE[key]


def kernel(self_feat, one_hop_feat, two_hop_feat, e_time, his_time,
           his_his_time, w1, b1, w2, b2, w3, b3, w4, b4, delta1):
    self_feat = np.ascontiguousarray(np.asarray(self_feat, dtype=np.float32))
    one_hop_feat = np.ascontiguousarray(np.asarray(one_hop_feat, dtype=np.float32))
    two_hop_feat = np.ascontiguousarray(np.asarray(two_hop_feat, dtype=np.float32))
    e_time = np.asarray(e_time, dtype=np.float32).reshape(B, 1)
    his_time = np.ascontiguousarray(np.asarray(his_time, dtype=np.float32))
    his_his_time = np.asarray(his_his_time, dtype=np.float32).reshape(B, H * H)
    ident, maskblk = make_const_inputs()
    shared = {
        "w1": np.asarray(w1, np.float32), "w2": np.asarray(w2, np.float32),
        "w3": np.asarray(w3, np.float32), "w4": np.asarray(w4, np.float32),
        "b1": np.asarray(b1, np.float32).reshape(1, F),
        "b2": np.asarray(b2, np.float32).reshape(1, F),
        "b3": np.asarray(b3, np.float32).reshape(1, F),
        "b4": np.asarray(b4, np.float32).reshape(1, F),
        "delta1": np.asarray(delta1, np.float32).reshape(1, 1),
        "ident": ident, "maskblk": maskblk,
    }
    in_maps = []
    for i in range(NCORES):
        ev = slice(i * BC, (i + 1) * BC)
        r1 = slice(i * BC * H, (i + 1) * BC * H)
        r2 = slice(i * BC * H * H, (i + 1) * BC * H * H)
        in_maps.append(dict(
            self_feat=self_feat[ev], one_hop=one_hop_feat[r1],
            two_hop=two_hop_feat[r2], e_time=e_time[ev],
            his_time=his_time[ev], his_his_time=his_his_time[ev], **shared))
    nc = _get_nc()
    res = run_bass_kernel_spmd(nc, in_maps, core_ids=list(range(NCORES)))
    return np.concatenate([res.results[i]["out"] for i in range(NCORES)], axis=0)

